# revision 4
# baseline (speedup 1.0000x reference)
"""CARAFE banded-matmul kernel, v4: 16-wide w-blocks, all 5 tap-rows stacked.

Each w-block covers 16 output columns; its source-column band is 20 wide.
All five tap-rows i stack along the contraction dim: partitions (i, u) =
i*20+u, K=100.  One matmul per (h, block): K=100, N=64 (sh,sw,w16), eight
blocks per h accumulating into one PSUM bank (start=True only on the first;
later blocks first-write into pending-zero columns, then accumulate).

vs v3: B traffic 5.9MB -> 3.28MB/core, PE column-streams halved (one K-pass
instead of two).  Features are i-replicated on the host (3.35MB, loaded once
outside the repeat loop).
"""

import numpy as np

N, C, H, W = 2, 128, 128, 128
K, S, R = 5, 2, 2
NT = K * K
HQ = 4
HPC = H // HQ
NCORES = 8
NG = 8
NB = 8        # w-blocks of 16
WB = 16       # block width
UB = WB + 4   # 20-wide band
KK = K * UB   # 100 contraction rows
FCOLS = 32 * 128
BC = 4 * NB * 4 * WB  # per-g B cols: t4 * b8 * (sh sw w16)=64 -> 2048

_prog_cache = {}


def _build_program(repeats=1, splitevac=True, outsp=False, unroll=16):
    import concourse.bacc as bacc
    import concourse.mybir as mybir
    from concourse.tile import TileContext
    import contextlib

    if repeats > 1:
        while repeats % unroll:
            unroll //= 2
        nloops = repeats // unroll
    else:
        unroll, nloops = 1, 1

    f32 = mybir.dt.float32
    bf16 = mybir.dt.bfloat16

    nc = bacc.Bacc(None, target_bir_lowering=False)
    fp = nc.dram_tensor("featS", [NB, KK, FCOLS], bf16, kind="ExternalInput")
    bd = nc.dram_tensor("bmat", [NG, KK, BC], bf16, kind="ExternalInput")
    out = nc.dram_tensor("out", [128, NG * 2048], bf16, kind="ExternalOutput")

    with TileContext(nc) as tc:
        with (
            tc.tile_pool(name="feat", bufs=1) as fpool,
            tc.tile_pool(name="bstream", bufs=5) as bpool,
            tc.tile_pool(name="stage", bufs=5) as spool,
            tc.tile_pool(name="acc", bufs=8, space="PSUM") as ppool,
        ):
            fts = []
            for b in range(NB):
                ft = fpool.tile([KK, FCOLS], bf16, tag=f"feat{b}")
                nc.sync.dma_start(out=ft[:], in_=fp[:][b])
                fts.append(ft[:].rearrange("u (r c) -> u r c", c=128))
            bdv = bd[:]

            rep_ctx = tc.For_i(0, nloops, 1) if repeats > 1 else contextlib.nullcontext()
            with rep_ctx:
                for _u in range(unroll):
                    for g in range(NG):
                        bt = bpool.tile([KK, BC], bf16, tag="bmat")
                        nc.sync.dma_start(out=bt[:], in_=bdv[g])
                        btv = bt[:].rearrange("u (t b c) -> u t b c", t=4, b=NB)
                        stage = spool.tile([128, 2048], bf16, tag="stage")
                        for t in range(4):
                            h = 4 * g + t
                            acc = ppool.tile([128, 512], f32)
                            for b in range(NB):
                                nc.tensor.matmul(
                                    acc[:, 64 * b : 64 * (b + 1)],
                                    lhsT=fts[b][:, h, :],
                                    rhs=btv[:, t, b],
                                    start=(b == 0),
                                    stop=(b == NB - 1),
                                    skip_group_check=True,
                                )
                            if splitevac and t % 2 == 1:
                                nc.vector.tensor_copy(
                                    stage[:, 512 * t : 512 * (t + 1)], acc[:]
                                )
                            else:
                                nc.scalar.copy(
                                    stage[:, 512 * t : 512 * (t + 1)], acc[:]
                                )
                        eng = nc.sync if outsp else nc.scalar
                        eng.dma_start(
                            out=out[:, 2048 * g : 2048 * (g + 1)], in_=stage[:]
                        )
    nc.finalize()
    return nc


def get_program(repeats=1):
    key = ("nc", repeats)
    if key not in _prog_cache:
        _prog_cache[key] = _build_program(repeats)
    return _prog_cache[key]


def _bf16(x):
    import ml_dtypes

    return np.ascontiguousarray(x).astype(ml_dtypes.bfloat16)


def make_in_maps(features, masks):
    features = np.asarray(features, dtype=np.float32)
    masks = np.asarray(masks, dtype=np.float32)

    in_maps = []
    for core in range(NCORES):
        n, q = divmod(core, HQ)
        h0 = HPC * q

        featpad = np.zeros((C, 36, W + 4), np.float32)
        lo = max(h0 - R, 0)
        hi = min(h0 + HPC + R, H)
        featpad[:, lo - (h0 - R) : hi - (h0 - R), 2 : 2 + W] = features[n, :, lo:hi, :]
        ft = featpad.transpose(2, 1, 0)  # [u_col 132, r 36, c 128]
        # block b, segment i: source cols 16b-2..16b+18 (padded coords 16b..16b+20),
        # rows h+i for h in 0..31 -> r slice i..i+32
        fS = np.stack(
            [
                np.concatenate(
                    [ft[16 * b : 16 * b + UB, i : i + 32, :] for i in range(K)]
                )
                for b in range(NB)
            ]
        )  # [8, 100, 32, 128]

        mk = masks[n, :, 2 * h0 : 2 * h0 + 2 * HPC, :]
        m8 = mk.reshape(NT, NG, 4, 2, NB, WB, 2)  # (tap, g, t, sh, b, w, sw)
        Z = np.zeros((NG, K, UB, 4, NB, 2, 2, WB), np.float32)  # (g,i,u,t,b,sh,sw,w)
        for i in range(K):
            for j in range(K):
                src = m8[K * i + j].transpose(0, 1, 4, 3, 2, 5)  # (g,t,w,b,sh,sw)
                for w in range(WB):
                    Z[:, i, w + j, :, :, :, :, w] = src[:, :, w].transpose(0, 1, 2, 3, 4)
        in_maps.append(
            {
                "featS": _bf16(fS.reshape(NB, KK, FCOLS)),
                "bmat": _bf16(Z.reshape(NG, KK, BC)),
            }
        )
    return in_maps


def gather_output(results):
    out = np.empty((N, C, 2 * H, 2 * W), np.float32)
    for core in range(NCORES):
        n, q = divmod(core, HQ)
        o = np.asarray(results[core]["out"], dtype=np.float32)
        o = o.reshape(C, NG, 4, NB, 2, 2, WB)  # (c, g, t, b, sh, sw, w)
        o = o.transpose(0, 1, 2, 4, 3, 6, 5)  # (c, g, t, sh, b, w, sw)
        out[n, :, 2 * HPC * q : 2 * HPC * (q + 1), :] = o.reshape(C, 2 * HPC, 2 * W)
    return out


def kernel(features, masks):
    from concourse.bass_utils import run_bass_kernel_spmd

    nc = get_program()
    in_maps = make_in_maps(features, masks)
    res = run_bass_kernel_spmd(nc, in_maps, core_ids=list(range(NCORES)))
    return gather_output(res.results)
